# revision 1
# baseline (speedup 1.0000x reference)
"""Trainium2 Bass kernel for nn_Decoder_74380243632630.

Decoder = LSTM-with-attention + vocab projection.  Key simplification:
the reference applies Softmax(dim=1) over a singleton axis, so the
attention score is identically 1.0 and the context vector is
z = enc_output.sum(axis=1), constant across time.  att1 / enc_att_W /
dec_att_W are dead code.

Per-core plan (replicated recurrence, vocab-sharded projection):
  z      = sum_L enc                      (selector matmul)
  G[t]   = emb[y_t] @ W_e^T + z @ W_z^T + b_ih + b_hh   (batched over t)
  LSTM   : gates_t = G[t] + h @ W_hh^T ; elementwise    (sequential, T=24)
  preds  = H @ vocab_W[shard]^T + vocab_b[shard]        (batched over t)

All matmuls run as float32r (fast fp32, ~1e-4 rel).  c-state stays fp32.
"""

import os
import sys
import threading

for _p in ("/opt/trn_rl_repo", "/root/.axon_site/_ro/trn_rl_repo"):
    if os.path.isdir(_p) and _p not in sys.path:
        sys.path.insert(0, _p)

import numpy as np
from contextlib import ExitStack

import concourse.bass as bass
import concourse.tile as tile
import concourse.mybir as mybir
from concourse import bacc
from concourse.bass_utils import run_bass_kernel_spmd

F32 = mybir.dt.float32
F32R = mybir.dt.float32r

# Problem dims (hardcoded per spec)
B, L, D = 32, 196, 512
T = 24
E, NH, V = 512, 1024, 32000
NC = 8
VS = V // NC          # 4000 vocab rows per core
G4 = 4 * NH           # 4096
NT = B * T            # 768 (row order t*32+b)
BL = B * L            # 6272 = 49*128
KL = BL // 128        # 49

# gate-type partition blocks: s=0:i, 1:f, 2:o, 3:g  (sigmoid on 0..95, tanh on 96..127)
# gate-vector column base per block (torch gate order is i,f,g,o)
TYPECOL = [0, 1 * NH, 3 * NH, 2 * NH]
S_OF_TYPE = [0, 1, 3, 2]   # gate type index (i,f,g,o) -> partition block


def emit_body(ctx, tc, aps, out_ap):
    """Emit the whole per-core program."""
    nc = tc.nc

    # ---------------- persistent pools ----------------
    small_pool = ctx.enter_context(tc.tile_pool(name="small", bufs=1))
    ht_pool = ctx.enter_context(tc.tile_pool(name="ht", bufs=1))
    gdram_pool = ctx.enter_context(tc.tile_pool(name="gdram", bufs=1, space="DRAM"))

    ident = small_pool.tile([128, 128], F32R)
    ones = small_pool.tile([1, 128], F32R)
    nc.sync.dma_start(ident[:], aps["ident"])
    nc.sync.dma_start(ones[:], aps["ones"])

    ht_acc = ht_pool.tile([128, 8, NT], F32R)     # H^T for vocab lhsT
    hT0 = small_pool.tile([128, 8, B], F32R)
    cbuf = small_pool.tile([B, NH], F32)

    # input gates staged in DRAM as [NT, hf, 4*512] with block-s column order
    g_dram = gdram_pool.tile([NT, 2, 2048], F32R)

    # W_hh chunks 0-2: pool outlives mid (LIFO), DMAs start immediately
    whh = []
    whh_early_ctx = ExitStack()
    whh_early = whh_early_ctx.enter_context(tc.tile_pool(name="whhE", bufs=1))
    for k in range(3):
        wt = whh_early.tile([128, G4], F32R, name=f"whh{k}")
        nc.sync.dma_start(wt[:], aps["w_hh"][128 * k:128 * (k + 1), :])
        whh.append(wt)

    # carried across phases A/B only
    mid_ctx = ExitStack()
    mid_pool = mid_ctx.enter_context(tc.tile_pool(name="mid", bufs=1))
    z_sb = mid_pool.tile([B, D], F32R)
    zT = mid_pool.tile([128, 4, B], F32R)
    gz_sb = mid_pool.tile([B, G4], F32R)
    meanT = mid_pool.tile([128, 4, B], F32R)

    # ---------------- phase A1: z = sum_L enc ----------------
    with ExitStack() as actx:
        a_pool = actx.enter_context(tc.tile_pool(name="phA1", bufs=1))
        a_psum = actx.enter_context(tc.tile_pool(name="phA1p", bufs=1, space="PSUM"))

        enc_sb = a_pool.tile([128, KL, D], F32R)
        sel_sb = a_pool.tile([128, KL, B], F32R)
        nc.sync.dma_start(enc_sb[:], aps["enc"].rearrange("(kt p) d -> p kt d", p=128))
        nc.sync.dma_start(sel_sb[:], aps["sel"].rearrange("(kt p) b -> p kt b", p=128))

        ps_z = a_psum.tile([B, D], F32)
        for k in range(KL):
            nc.tensor.matmul(ps_z[:], sel_sb[:, k, :], enc_sb[:, k, :],
                             start=(k == 0), stop=(k == KL - 1))
        nc.vector.tensor_copy(z_sb[:], ps_z[:])

        ps_zt = a_psum.tile([128, 4, B], F32R)
        for j in range(4):
            nc.tensor.transpose(ps_zt[:, j, :], z_sb[:, 128 * j:128 * (j + 1)],
                                ident[0:B, 0:B])
        nc.vector.tensor_copy(zT[:], ps_zt[:])

    # ---------------- phase A2a: G_z ----------------
    with ExitStack() as actx:
        a_pool = actx.enter_context(tc.tile_pool(name="phA2", bufs=1))

        nc.vector.tensor_scalar_mul(meanT[:], zT[:], 1.0 / L)

        wz_sb = a_pool.tile([128, 4, G4], F32R)
        nc.sync.dma_start(wz_sb[:], aps["w_z"].rearrange("(kt p) g -> p kt g", p=128))
        bias_g = a_pool.tile([1, G4], F32R)
        nc.sync.dma_start(bias_g[:], aps["bias_g"])

        with ExitStack() as pctx:
            a_psum = pctx.enter_context(tc.tile_pool(name="phA2p", bufs=2, space="PSUM"))
            for half in range(2):
                ps_gz = a_psum.tile([B, 2048], F32, name=f"psgz{half}", tag="psgz")
                for n in range(4):
                    nn_ = 4 * half + n
                    sl = slice(512 * n, 512 * (n + 1))
                    gsl = slice(512 * nn_, 512 * (nn_ + 1))
                    for k in range(4):
                        nc.tensor.matmul(ps_gz[:, sl], zT[:, k, :], wz_sb[:, k, gsl],
                                         start=(k == 0), stop=False)
                    nc.tensor.matmul(ps_gz[:, sl], ones[0:1, 0:B], bias_g[0:1, gsl],
                                     start=False, stop=True)
                nc.vector.tensor_copy(gz_sb[:, 2048 * half:2048 * (half + 1)], ps_gz[:])

    # ---------------- phase A2b: h0/c0 ----------------
    with ExitStack() as actx:
        a_pool = actx.enter_context(tc.tile_pool(name="phA2b", bufs=1))
        ih_sb = a_pool.tile([128, 4, NH], F32R)
        ic_sb = a_pool.tile([128, 4, NH], F32R)
        nc.sync.dma_start(ih_sb[:], aps["init_h_w"].rearrange("(kt p) n -> p kt n", p=128))
        nc.sync.dma_start(ic_sb[:], aps["init_c_w"].rearrange("(kt p) n -> p kt n", p=128))
        bias_h = a_pool.tile([1, NH], F32R)
        bias_c = a_pool.tile([1, NH], F32R)
        nc.sync.dma_start(bias_h[:], aps["bias_h"])
        nc.sync.dma_start(bias_c[:], aps["bias_c"])

        with ExitStack() as pctx:
            a_psum = pctx.enter_context(tc.tile_pool(name="phA2q", bufs=1, space="PSUM"))
            ps_h0 = a_psum.tile([B, NH], F32)
            ps_c0 = a_psum.tile([B, NH], F32)
            for n in range(2):
                sl = slice(512 * n, 512 * (n + 1))
                for k in range(4):
                    nc.tensor.matmul(ps_h0[:, sl], meanT[:, k, :], ih_sb[:, k, sl],
                                     start=(k == 0), stop=False)
                nc.tensor.matmul(ps_h0[:, sl], ones[0:1, 0:B], bias_h[0:1, sl],
                                 start=False, stop=True)
                for k in range(4):
                    nc.tensor.matmul(ps_c0[:, sl], meanT[:, k, :], ic_sb[:, k, sl],
                                     start=(k == 0), stop=False)
                nc.tensor.matmul(ps_c0[:, sl], ones[0:1, 0:B], bias_c[0:1, sl],
                                 start=False, stop=True)
            h_sb = a_pool.tile([B, NH], F32R)
            nc.vector.tensor_copy(h_sb[:], ps_h0[:])
            nc.vector.tensor_copy(cbuf[:], ps_c0[:])

            ps_ht0 = a_psum.tile([128, 8, B], F32R)
            for k in range(8):
                nc.tensor.transpose(ps_ht0[:, k, :], h_sb[:, 128 * k:128 * (k + 1)],
                                    ident[0:B, 0:B])
            nc.vector.tensor_copy(hT0[:], ps_ht0[:])

    # ---------------- phase B: batched input gates G ----------------
    with ExitStack() as bctx:
        b_pool = bctx.enter_context(tc.tile_pool(name="phB", bufs=1))
        b_out = bctx.enter_context(tc.tile_pool(name="phBo", bufs=3))
        b_psum = bctx.enter_context(tc.tile_pool(name="phBp", bufs=2, space="PSUM"))

        x2a = b_pool.tile([128, 4, NT], F32R)
        x2b = b_pool.tile([B, NT], F32R)
        we_sb = b_pool.tile([128, 4, G4], F32R)
        nc.sync.dma_start(x2a[:], aps["x2a"].rearrange("(kt p) r -> p kt r", p=128))
        nc.sync.dma_start(x2b[:], aps["x2b"])
        nc.sync.dma_start(we_sb[:], aps["w_e"].rearrange("(kt p) g -> p kt g", p=128))

        for mi in range(6):
            msl = slice(128 * mi, 128 * (mi + 1))
            for n in range(8):
                gsl = slice(512 * n, 512 * (n + 1))
                tt, hf = n // 2, n % 2
                s = S_OF_TYPE[tt]
                ps_G = b_psum.tile([128, 512], F32, name=f"psG{mi}_{n}", tag="psG")
                for k in range(4):
                    nc.tensor.matmul(ps_G[:], x2a[:, k, msl], we_sb[:, k, gsl],
                                     start=(k == 0), stop=False)
                nc.tensor.matmul(ps_G[:], x2b[:, msl], gz_sb[:, gsl],
                                 start=False, stop=True)
                g_out = b_out.tile([128, 512], F32R, name=f"go{mi}_{n}", tag="gout")
                nc.vector.tensor_copy(g_out[:], ps_G[:])
                nc.sync.dma_start(g_dram[msl, hf, 512 * s:512 * (s + 1)], g_out[:])

    mid_ctx.close()

    # ---------------- phase C: recurrence ----------------
    with ExitStack() as cctx:
        whh_pool = cctx.enter_context(tc.tile_pool(name="whhL", bufs=1))
        g_pool = cctx.enter_context(tc.tile_pool(name="phCg", bufs=1))
        e1_pool = cctx.enter_context(tc.tile_pool(name="phCe1", bufs=1))
        c_psum = cctx.enter_context(tc.tile_pool(name="phCp", bufs=2, space="PSUM"))

        for k in range(3, 8):
            wt = whh_pool.tile([128, G4], F32R, name=f"whh{k}")
            nc.sync.dma_start(wt[:], aps["w_hh"][128 * k:128 * (k + 1), :])
            whh.append(wt)

        for t in range(T):
            g_t = [g_pool.tile([B, 2048], F32R, name=f"g{t}_{hf}", tag=f"g_hf{hf}")
                   for hf in range(2)]
            for hf in range(2):
                nc.sync.dma_start(g_t[hf][:], g_dram[B * t:B * (t + 1), hf, :])

            # gates psum per hf-half: free cols = [i | f | o | g] 512 each
            ps = [c_psum.tile([B, 2048], F32, name=f"psg{t}_{hf}", tag="ps")
                  for hf in range(2)]
            for hf in range(2):
                for k in range(8):
                    lt = hT0[:, k, :] if t == 0 else ht_acc[:, k, B * (t - 1):B * t]
                    for s in range(4):
                        nc.tensor.matmul(
                            ps[hf][:, 512 * s:512 * (s + 1)],
                            lt, whh[k][:, TYPECOL[s] + 512 * hf:TYPECOL[s] + 512 * (hf + 1)],
                            start=(k == 0), stop=False)
                for s in range(4):
                    nc.tensor.matmul(
                        ps[hf][:, 512 * s:512 * (s + 1)],
                        ident[0:B, 0:B], g_t[hf][:, 512 * s:512 * (s + 1)],
                        start=False, stop=True)

            # elementwise LSTM cell on [B, 2, 2048] views: cols [i|f|o|g] per hf
            sig = e1_pool.tile([B, 2, 2048], F32, name=f"sig{t}", tag="sig")
            for hf in range(2):
                nc.scalar.activation(sig[:, hf, 0:1536], ps[hf][:, 0:1536],
                                     mybir.ActivationFunctionType.Sigmoid)
                nc.scalar.activation(sig[:, hf, 1536:2048], ps[hf][:, 1536:2048],
                                     mybir.ActivationFunctionType.Tanh)
            si = sig[:, :, 0:512]
            sf = sig[:, :, 512:1024]
            so = sig[:, :, 1024:1536]
            sg = sig[:, :, 1536:2048]
            cv = cbuf[:].rearrange("b (hf c) -> b hf c", hf=2)
            # t1 = i*g -> si ; t2 = f*c -> sf ; c_new = t1+t2 -> cbuf
            nc.vector.tensor_mul(si, si, sg)
            nc.vector.tensor_mul(sf, sf, cv)
            nc.vector.tensor_add(cv, si, sf)
            # thc = tanh(c_new) -> sg ; h = o * thc
            nc.scalar.activation(sg, cv, mybir.ActivationFunctionType.Tanh)
            h_new = e1_pool.tile([B, NH], F32R, name=f"h{t}", tag="h_new")
            nc.vector.tensor_mul(h_new[:].rearrange("b (hf c) -> b hf c", hf=2), so, sg)

            ps_ht = c_psum.tile([128, 8, B], F32R, name=f"psht{t}", tag="ps")
            for k in range(8):
                nc.tensor.transpose(ps_ht[:, k, :], h_new[:, 128 * k:128 * (k + 1)],
                                    ident[0:B, 0:B])
            nc.vector.tensor_copy(ht_acc[:, :, B * t:B * (t + 1)], ps_ht[:])

    whh_early_ctx.close()

    # ---------------- phase D: vocab projection ----------------
    with ExitStack() as dctx:
        d_pool = dctx.enter_context(tc.tile_pool(name="phD", bufs=2))
        d_out = dctx.enter_context(tc.tile_pool(name="phDo", bufs=3))
        d_psum = dctx.enter_context(tc.tile_pool(name="phDp", bufs=2, space="PSUM"))

        NV = VS // 8  # 500
        for n in range(8):
            vsl = slice(NV * n, NV * (n + 1))
            vw = d_pool.tile([128, 8, NV], F32R, name=f"vw{n}", tag="vw")
            nc.sync.dma_start(vw[:], aps["vwt"][:, vsl].rearrange("(kt p) v -> p kt v", p=128))
            vb = d_pool.tile([1, NV], F32R, name=f"vb{n}", tag="vb")
            nc.sync.dma_start(vb[:], aps["vb"][:, vsl])
            for mi in range(6):
                msl = slice(128 * mi, 128 * (mi + 1))
                ps_p = d_psum.tile([128, NV], F32, name=f"psp{n}_{mi}", tag="psp")
                for k in range(8):
                    nc.tensor.matmul(ps_p[:], ht_acc[:, k, msl], vw[:, k, :],
                                     start=(k == 0), stop=False)
                nc.tensor.matmul(ps_p[:], ones[0:1, :], vb[0:1, :],
                                 start=False, stop=True)
                p_out = d_out.tile([128, NV], F32, name=f"po{n}_{mi}", tag="pout")
                nc.vector.tensor_copy(p_out[:], ps_p[:])
                nc.sync.dma_start(out_ap[msl, vsl], p_out[:])


def build_program(rep_loop=None):
    """Build the Bass program.  rep_loop: if an int > 1, wrap the body in a
    dynamic For_i for hardware timing."""
    nc = bacc.Bacc("TRN2", target_bir_lowering=False, debug=False)

    aps = {}
    def din(name, shape, dt=F32R):
        aps[name] = nc.dram_tensor(name, shape, dt, kind="ExternalInput").ap()

    din("enc", [BL, D])
    din("sel", [BL, B])
    din("x2a", [E, NT])
    din("x2b", [B, NT])
    din("w_e", [E, G4])
    din("w_z", [D, G4])
    din("w_hh", [NH, G4])
    din("init_h_w", [D, NH])
    din("init_c_w", [D, NH])
    din("bias_g", [1, G4])
    din("bias_h", [1, NH])
    din("bias_c", [1, NH])
    din("vwt", [NH, VS])
    din("vb", [1, VS])
    din("ident", [128, 128])
    din("ones", [1, 128])

    out_ap = nc.dram_tensor("preds", [NT, VS], F32, kind="ExternalOutput").ap()

    trace_sim = bool(os.environ.get("KERNEL_TRACE_SIM"))
    with tile.TileContext(nc, trace_sim=trace_sim) as tc:
        with ExitStack() as ctx:
            if rep_loop is not None and rep_loop > 1:
                with tc.For_i(0, rep_loop, 1):
                    emit_body(ctx, tc, aps, out_ap)
            else:
                emit_body(ctx, tc, aps, out_ap)
    nc.compile()
    return nc


def host_prep(inputs):
    """Slice/transpose full inputs into the 8 per-core input maps."""
    f32 = np.float32
    enc_output = np.asarray(inputs["enc_output"], dtype=f32)
    y = np.asarray(inputs["y"])
    emb_table = np.asarray(inputs["emb_table"], dtype=f32)
    W_ih = np.asarray(inputs["W_ih"], dtype=f32)
    W_hh = np.asarray(inputs["W_hh"], dtype=f32)
    b_ih = np.asarray(inputs["b_ih"], dtype=f32)
    b_hh = np.asarray(inputs["b_hh"], dtype=f32)
    init_h_W = np.asarray(inputs["init_h_W"], dtype=f32)
    init_h_b = np.asarray(inputs["init_h_b"], dtype=f32)
    init_c_W = np.asarray(inputs["init_c_W"], dtype=f32)
    init_c_b = np.asarray(inputs["init_c_b"], dtype=f32)
    vocab_W = np.asarray(inputs["vocab_W"], dtype=f32)
    vocab_b = np.asarray(inputs["vocab_b"], dtype=f32)

    common = {}
    common["enc"] = np.ascontiguousarray(enc_output.reshape(BL, D))
    sel = np.zeros((BL, B), dtype=f32)
    for b in range(B):
        sel[b * L:(b + 1) * L, b] = 1.0
    common["sel"] = sel
    # emb_x[b, t] = emb_table[y[b, t]]; cols ordered t*32+b
    emb_x = emb_table[y]                       # [B, T, E]
    common["x2a"] = np.ascontiguousarray(emb_x.transpose(2, 1, 0).reshape(E, NT))
    common["x2b"] = np.ascontiguousarray(np.tile(np.eye(B, dtype=f32), (1, T)))
    common["w_e"] = np.ascontiguousarray(W_ih[:, :E].T)
    common["w_z"] = np.ascontiguousarray(W_ih[:, E:].T)
    common["w_hh"] = np.ascontiguousarray(W_hh.T)
    common["init_h_w"] = np.ascontiguousarray(init_h_W.T)
    common["init_c_w"] = np.ascontiguousarray(init_c_W.T)
    common["bias_g"] = (b_ih + b_hh).reshape(1, G4)
    common["bias_h"] = init_h_b.reshape(1, NH)
    common["bias_c"] = init_c_b.reshape(1, NH)
    common["ident"] = np.eye(128, dtype=f32)
    common["ones"] = np.ones((1, 128), dtype=f32)

    in_maps = []
    for p in range(NC):
        m = dict(common)
        m["vwt"] = np.ascontiguousarray(vocab_W[VS * p:VS * (p + 1), :].T)
        m["vb"] = vocab_b[VS * p:VS * (p + 1)].reshape(1, VS)
        in_maps.append(m)
    return in_maps


def assemble_output(results):
    full = np.empty((B, V, T), dtype=np.float32)
    for p in range(NC):
        r = results[p]["preds"].reshape(T, B, VS)
        full[:, VS * p:VS * (p + 1), :] = r.transpose(1, 2, 0)
    return full


_cache = threading.Lock(), {}


def _get_program():
    lock, cache = _cache
    with lock:
        if "nc" not in cache:
            cache["nc"] = build_program()
        return cache["nc"]


def kernel(**inputs):
    nc = _get_program()
    in_maps = host_prep(inputs)
    res = run_bass_kernel_spmd(nc, in_maps, core_ids=list(range(NC)))
    return assemble_output(res.results)


if __name__ == "__main__":
    print("building program...")
    import time
    t0 = time.time()
    nc = _get_program()
    print(f"build+compile: {time.time()-t0:.1f}s")

